# revision 1
# baseline (speedup 1.0000x reference)
"""Trainium2 Bass kernel for nn_DijkstraGNN (5-layer scatter-min GNN).

Self-contained: host-side graph preprocessing (dst-sharded, degree-bucketed
duplicate-padding), an SPMD Bass/Tile program for 8 NeuronCores, and output
assembly. Called as kernel(**inputs) with the full unsharded inputs.
"""
import sys
sys.path.insert(0, "/opt/trn_rl_repo")
import numpy as np
import concourse.bass as bass
import concourse.mybir as mybir
from concourse.tile import TileContext

# ===================== host-side preprocessing =====================
"""Host-side graph preprocessing for the dst-sharded, bucket-padded GNN kernel.

Layout (per core, 12500 owned nodes, halves A/B of 6250):
- Edge "slots" live at positions (p=partition, u=column) of the gather tile.
  Transpose chunk a covers u in {2a, 2a+1}; u even -> half A (psum rows 0-63),
  u odd -> half B (rows 64-127). PSUM column (within half) = 128*(u//2) + p... i.e.
  col c <-> (p = c % 128, u = 2*(c//128) + half).
- A node's K slots occupy K consecutive psum columns; runs never straddle a
  psum tile. Tiles are uniform-K, width = 384 cols (K % 3 == 0) else 512.
- K in {4,8,12,16,24,32,48,64,96,128}. Buckets laid out K-DESCENDING; the final
  (K=4) bucket is ghost-padded at the end so per-half chunk counts are integral.
- aggr col order defines the node permutation: half A cols = local rows 0..6249,
  half B = 6250..12499. Geometry (tiles list) is identical across cores.
"""

N = 100000
NCORES = 8
NODES_PER_CORE = N // NCORES      # 12500
HALF = NODES_PER_CORE // 2        # 6250
H = 64
CHUNK = 128

KS_DESC = [128, 96, 64, 48, 32, 24, 16, 12, 8]
K_MIN = KS_DESC[-1]


def width_of(K):
    return 384 if K % 3 == 0 else 512


def k_of_degree(d):
    for K in reversed(KS_DESC):           # ascending
        if d <= K:
            return K
    raise ValueError(f"degree {d} > 128 unsupported")


def prep(edge_index, edge_attr, x, n=None, ncores=None):
    global N, NCORES, NODES_PER_CORE, HALF
    if n is not None:
        N = n
    if ncores is not None:
        NCORES = ncores
    NODES_PER_CORE = N // NCORES
    HALF = NODES_PER_CORE // 2
    src = np.asarray(edge_index[0]).astype(np.int64).reshape(-1)
    dst = np.asarray(edge_index[1]).astype(np.int64).reshape(-1)
    ea = np.asarray(edge_attr).reshape(-1).astype(np.float32)
    xv = np.asarray(x).reshape(-1).astype(np.float32)

    deg = np.bincount(dst, minlength=N)
    assert deg.min() >= 1, "degree-0 nodes unsupported (none in this dataset)"

    order = np.argsort(dst, kind="stable")
    ptr = np.zeros(N + 1, dtype=np.int64)
    np.cumsum(deg, out=ptr[1:])

    K_node = np.array([k_of_degree(d) for d in deg], dtype=np.int64)

    # --- equalize bucket node counts across cores (K descending) ---
    # C[r][i] = nodes of core r with K_node >= KS_DESC[i]  (cumulative)
    Cs = np.zeros((NCORES, len(KS_DESC)), dtype=np.int64)
    for r in range(NCORES):
        kn = K_node[r * NODES_PER_CORE:(r + 1) * NODES_PER_CORE]
        for i, K in enumerate(KS_DESC):
            Cs[r, i] = (kn >= K).sum()
    maxreq = Cs.max(axis=0)

    target = {}
    cum = 0
    for i, K in enumerate(KS_DESC[:-1]):
        per_half_nodes_unit = 1
        # per-half col count must be a multiple of CHUNK => count*K/2 % 128 == 0
        unit = 2 * CHUNK // np.gcd(K, 2 * CHUNK)          # count unit
        unit = max(unit, 2)                               # even halves
        # also ensure per-half count tiles into uniform-K tiles exactly:
        w = width_of(K)
        unit2 = 2 * (w // K) // np.gcd(w // K, 1)
        # count/2 % (w/K) == 0  => count % (2*w/K) == 0 ; and chunk alignment
        # (count/2)*K % 128 == 0 is implied when w % 128 == 0 and count/2 % (w/K)==0
        unit = int(np.lcm(unit, 2 * (w // K)))
        raw = max(0, int(maxreq[i]) - cum)
        t = ((raw + unit - 1) // unit) * unit
        target[K] = t
        cum += t
    target[K_MIN] = NODES_PER_CORE - cum                  # ghost-padded below

    def feasible():
        # cumulative-from-top coverage must meet maxreq at every level
        c = 0
        for i, K in enumerate(KS_DESC):
            c += target.get(K, 0)
            if c < maxreq[i]:
                return False
        return all(v >= 0 for v in target.values())

    # rounding may have over-allocated; shrink upper buckets (by their units)
    # while keeping coverage, until the last bucket is non-negative & feasible
    units = {}
    for K in KS_DESC[:-1]:
        w = width_of(K)
        units[K] = int(np.lcm(2 * CHUNK // np.gcd(K, 2 * CHUNK), 2 * (w // K)))
    guard = 0
    while not feasible():
        guard += 1
        assert guard < 10000, "bucket balancing failed"
        fixed = False
        for K in KS_DESC[-2::-1]:                 # ascending, skip K_MIN
            if target.get(K, 0) >= units[K]:
                target[K] -= units[K]
                target[K_MIN] += units[K]
                if feasible():
                    fixed = True
                    break
                # revert if this broke coverage at upper levels
                target[K] += units[K]
                target[K_MIN] -= units[K]
        if not fixed:
            raise RuntimeError("could not balance bucket targets")
    assert sum(target.values()) == NODES_PER_CORE

    # ghost pad for the last bucket: per-half COLUMN count must be a multiple
    # of CHUNK=128 (transpose granularity).
    c4 = target[K_MIN]
    assert c4 % 2 == 0, "last bucket count must be even"
    ghosts_half = ((-(c4 // 2) * K_MIN) % CHUNK) // K_MIN  # ghost nodes per half
    assert ((c4 // 2) + ghosts_half) * K_MIN % CHUNK == 0
    # --- tiles (uniform geometry across cores) ---
    tiles = []            # (K, ncols) per psum tile, ncols = per-half cols
    chunk_plan = []       # (K, per-half cols) per bucket, K descending
    for K in KS_DESC:
        t = target.get(K, 0)
        if K == K_MIN:
            cols = (t // 2 + ghosts_half) * K
        else:
            cols = t // 2 * K
        if cols == 0:
            continue
        chunk_plan.append((K, cols))
        w = width_of(K)
        nfull, rem = divmod(cols, w)
        tiles.extend([(K, w)] * nfull)
        if rem:
            assert rem % CHUNK == 0
            tiles.append((K, rem))
    S_half_cols = sum(c for _, c in chunk_plan)
    assert S_half_cols % CHUNK == 0
    S = 2 * S_half_cols

    perm = np.empty(N, dtype=np.int64)
    cores = []
    rng = np.random.default_rng(0)
    for r in range(NCORES):
        nodes = np.arange(r * NODES_PER_CORE, (r + 1) * NODES_PER_CORE)
        ksort = np.argsort(-K_node[nodes], kind="stable")  # descending K
        nodes = nodes[ksort]
        pos = 0
        slot_src = np.zeros((128, S // 128), dtype=np.int32)
        ea_rank = np.zeros((2, S_half_cols), dtype=np.float32)
        x_rank = np.zeros((2, S_half_cols), dtype=np.float32)
        permA, permB = [], []
        colbase = 0
        for K, cols in chunk_plan:
            nreal = target[K] // 2                 # real nodes per half
            for h_i in range(2):
                sel = nodes[pos + h_i * nreal: pos + (h_i + 1) * nreal]
                plist = permA if h_i == 0 else permB
                plist.extend(sel.tolist())
                # vectorized slot fill for this (bucket, half)
                if len(sel) == 0 and K != 4:
                    continue
                # edge ids per node, duplicate-padded to K
                eidx = np.zeros((max(len(sel), 1), K), dtype=np.int64)
                for j, node in enumerate(sel):
                    es = order[ptr[node]:ptr[node + 1]]
                    eidx[j] = np.resize(es, K)
                if len(sel) == 0:
                    eidx = eidx[:0]
                nghost = (cols - nreal * K) // K if K == K_MIN else 0
                if nghost:
                    ghost = np.zeros((nghost, K), dtype=np.int64)
                    ghost[:] = order[0]            # arbitrary real edge
                    eidx = np.concatenate([eidx, ghost], axis=0) if len(eidx) \
                        else ghost
                flat = eidx.reshape(-1)            # per-half col order
                cvec = colbase + np.arange(len(flat))
                pvec = cvec % CHUNK
                uvec = 2 * (cvec // CHUNK) + h_i
                slot_src[pvec, uvec] = src[flat]   # orig node id; remapped below
                ea_rank[h_i, colbase:colbase + len(flat)] = ea[flat]
                x_rank[h_i, colbase:colbase + len(flat)] = xv[src[flat]]
            pos += target[K]
            colbase += cols
        assert pos == NODES_PER_CORE and colbase == S_half_cols
        assert len(permA) == len(permB) == HALF
        perm_local = np.array(permA + permB, dtype=np.int64)
        perm[r * NODES_PER_CORE:(r + 1) * NODES_PER_CORE] = perm_local
        cores.append({"perm_local": perm_local, "slot_src": slot_src,
                      "ea_rank": ea_rank, "x_rank": x_rank})

    inv_perm = np.empty(N, dtype=np.int64)
    inv_perm[perm] = np.arange(N)
    # remap slot_src from orig node ids to permuted row ids
    for c in cores:
        c["slot_src"] = inv_perm[c["slot_src"]].astype(np.int32)

    return {"tiles": tiles, "S": S, "chunk_plan": chunk_plan, "cores": cores,
            "perm": perm, "inv_perm": inv_perm, "deg": deg,
            "S_half_cols": S_half_cols,
            "n_aggr_cols": sum(c // K for K, c in chunk_plan)}


# ===================== input packing =====================
"""Host-side input packing shared by test and kernel.py."""

H = 64


def make_inmaps(inputs, pp, npc):
    """Build per-core in_maps for the bass kernel."""
    L = inputs["msg_w1"].shape[0]
    W1 = np.asarray(inputs["msg_w1"], np.float32)       # [L, 65, 64]
    b1 = np.asarray(inputs["msg_b1"], np.float32)
    W2 = np.asarray(inputs["msg_w2"], np.float32)
    b2 = np.asarray(inputs["msg_b2"], np.float32)
    U1 = np.asarray(inputs["upd_w1"], np.float32)       # [L, 128, 64]
    c1 = np.asarray(inputs["upd_b1"], np.float32)
    U2 = np.asarray(inputs["upd_w2"], np.float32)
    c2 = np.asarray(inputs["upd_b2"], np.float32)
    emb_w = np.asarray(inputs["emb_w"], np.float32)     # [1, 64]
    emb_b = np.asarray(inputs["emb_b"], np.float32)     # [64]
    fc_w = np.asarray(inputs["fc_w"], np.float32)       # [64, 1]
    fc_b = np.asarray(inputs["fc_b"], np.float32)       # [1]
    xv = np.asarray(inputs["x"], np.float32).reshape(-1)

    W1a = W1[:, :H]                                     # [L, 64, 64]
    w1b = W1[:, H]                                      # [L, 64]
    alpha = emb_w[0] @ W1a[0]
    beta = emb_b @ W1a[0] + b1[0]

    l0w = np.zeros((4, 128), np.float32)
    l0w[0, :64] = alpha; l0w[1, :64] = w1b[0]
    l0w[2, 64:] = alpha; l0w[3, 64:] = w1b[0]
    eaw = np.zeros((L - 1, 2, 128), np.float32)
    for l in range(1, L):
        eaw[l - 1, 0, :64] = w1b[l]
        eaw[l - 1, 1, 64:] = w1b[l]
    b1r = np.zeros((L, 128, 1), np.float32)
    b1r[0, :, 0] = np.tile(beta, 2)
    for l in range(1, L):
        b1r[l, :, 0] = np.tile(b1[l], 2)
    w2s = np.zeros((L, 128, 128), np.float32)           # block-diag W2
    w2s[:, :64, :64] = W2
    w2s[:, 64:, 64:] = W2
    u1a = np.concatenate([U1[:, H:], U1[:, :H]], axis=1)
    u1b = U1
    c1p = c1 + np.einsum("lk,lko->lo", b2, U1[:, H:])
    c1r = np.tile(c1p[:, :, None], (1, 2, 1)).reshape(L, 128, 1)
    c2r = np.tile(c2[:, :, None], (1, 2, 1)).reshape(L, 128, 1)
    w1n = np.zeros((L - 1, 128, 128), np.float32)       # anti-block W1a_{l+1}
    w1n[:, 64:, :64] = W1a[1:]                          # A: h@64-127 -> gs@0-63
    w1n[:, :64, 64:] = W1a[1:]                          # B: h@0-63 -> gs@64-127
    fcw = np.zeros((128, 2), np.float32)
    fcw[64:, 0] = fc_w[:, 0]                            # A variant
    fcw[:64, 1] = fc_w[:, 0]                            # B variant
    fcb = fc_b.reshape(1, 1)
    emb2 = np.zeros((2, 256), np.float32)               # [2, 2*128] variants
    emb2[0, 64:128] = emb_w[0]; emb2[1, 64:128] = emb_b     # A -> rows 64-127
    emb2[0, 128:192] = emb_w[0]; emb2[1, 128:192] = emb_b   # B -> rows 0-63
    ident = np.eye(128, dtype=np.float32)
    u2v = np.zeros((L, 64, 256), np.float32)
    u2v[:, :, 64:128] = U2                              # A -> out rows 64-127
    u2v[:, :, 128:192] = U2                             # B -> out rows 0-63

    def pack(a):      # [L, P, C] -> [P, L*C]
        return np.ascontiguousarray(a.transpose(1, 0, 2).reshape(a.shape[1], -1))

    shared = dict(l0w=l0w, eaw=pack(eaw), b1r=pack(b1r), w2s=pack(w2s),
                  u1a=pack(u1a), u1b=pack(u1b), c1r=pack(c1r), c2r=pack(c2r),
                  u2=pack(u2v), w1n=pack(w1n), fcw=fcw, fcb=fcb,
                  emb2=emb2, ident=ident,
                  ident2=np.concatenate(
                      [np.concatenate([np.eye(64), np.zeros((64, 64))], 0),
                       np.concatenate([np.zeros((64, 64)), np.eye(64)], 0)],
                      axis=1).astype(np.float32))

    in_maps = []
    for r, core in enumerate(pp["cores"]):
        m = dict(shared)
        m["idx"] = core["slot_src"]
        m["ea2"] = core["ea_rank"]
        m["xe4"] = np.concatenate([core["x_rank"][0:1], core["ea_rank"][0:1],
                                   core["x_rank"][1:2], core["ea_rank"][1:2]],
                                  axis=0)
        xo = xv[core["perm_local"]]
        m["xo2"] = np.stack([xo, np.ones(npc, np.float32)], axis=0)
        in_maps.append(m)
    return in_maps


def assemble_output(results, pp, n, npc):
    out = np.zeros(n, dtype=np.float32)
    for r in range(len(results)):
        loc = results[r]["out_y"].reshape(-1)
        out[pp["perm"][r * npc:(r + 1) * npc]] = loc
    return out


# ===================== device program =====================
"""Bass device-program builder for the DijkstraGNN kernel."""

F32 = mybir.dt.float32
F32R = mybir.dt.float32r
I32 = mybir.dt.int32
AF = mybir.ActivationFunctionType
ALU = mybir.AluOpType


def build_nc(geom, L=5, n_nodes_total=100000, npc=12500, ncores=8, debug=False):
    S = geom["S"]
    S_half = geom["S_half_cols"]
    tiles = geom["tiles"]          # list of (K, w) psum tiles, uniform-K
    HALFN = npc // 2
    n_aggr = geom["n_aggr_cols"]

    GT = 48                        # idx-cols per gather group
    groups = []                    # [ucol0, nucols, [tile indices]]
    cur = None
    ucum = 0
    for ti, (K, w) in enumerate(tiles):
        uc = w // 64
        if cur is None or cur[1] + uc > GT:
            cur = [ucum, 0, []]
            groups.append(cur)
        cur[1] += uc
        cur[2].append(ti)
        ucum += uc
    assert ucum == S // 128

    nc = bass.Bass(num_swdge_queues=4)
    idx_d = nc.dram_tensor("idx", [128, S // 128], I32, kind="ExternalInput")
    ea2_d = nc.dram_tensor("ea2", [2, S_half], F32R, kind="ExternalInput")
    xe4_d = nc.dram_tensor("xe4", [4, S_half], F32R, kind="ExternalInput")
    xo2_d = nc.dram_tensor("xo2", [2, npc], F32R, kind="ExternalInput")
    emb2_d = nc.dram_tensor("emb2", [2, 256], F32R, kind="ExternalInput")
    l0w_d = nc.dram_tensor("l0w", [4, 128], F32R, kind="ExternalInput")
    eaw_d = nc.dram_tensor("eaw", [2, (L - 1) * 128], F32R, kind="ExternalInput")
    b1r_d = nc.dram_tensor("b1r", [128, L], F32, kind="ExternalInput")
    w2s_d = nc.dram_tensor("w2s", [128, L * 128], F32R, kind="ExternalInput")
    u1a_d = nc.dram_tensor("u1a", [128, L * 64], F32R, kind="ExternalInput")
    u1b_d = nc.dram_tensor("u1b", [128, L * 64], F32R, kind="ExternalInput")
    c1r_d = nc.dram_tensor("c1r", [128, L], F32, kind="ExternalInput")
    c2r_d = nc.dram_tensor("c2r", [128, L], F32, kind="ExternalInput")
    u2_d = nc.dram_tensor("u2", [64, L * 256], F32R, kind="ExternalInput")
    w1n_d = nc.dram_tensor("w1n", [128, (L - 1) * 128], F32R, kind="ExternalInput")
    fcw_d = nc.dram_tensor("fcw", [128, 2], F32R, kind="ExternalInput")
    fcb_d = nc.dram_tensor("fcb", [1, 1], F32, kind="ExternalInput")
    ident_d = nc.dram_tensor("ident", [128, 128], F32, kind="ExternalInput")
    ident2_d = nc.dram_tensor("ident2", [128, 128], F32R, kind="ExternalInput")
    out_d = nc.dram_tensor("out_y", [1, npc], F32, kind="ExternalOutput")
    if debug:
        dbg_h1 = nc.dram_tensor("dbg_h1", [128, npc // 2], F32, kind="ExternalOutput")
        dbg_ag = nc.dram_tensor("dbg_ag", [128, geom["n_aggr_cols"]], F32, kind="ExternalOutput")
        dbg_h2 = nc.dram_tensor("dbg_h2", [128, npc // 2], F32, kind="ExternalOutput")
        dbg_go = nc.dram_tensor("dbg_go", [npc, 64], F32, kind="ExternalOutput")
        dbg_g2 = nc.dram_tensor("dbg_g2", [n_nodes_total, 64], F32, kind="ExternalOutput")
        dbg_ag1 = nc.dram_tensor("dbg_ag1", [128, geom["n_aggr_cols"]], F32, kind="ExternalOutput")
        dbg_gt = nc.dram_tensor("dbg_gt", [128, GT * 64], F32, kind="ExternalOutput")
        dbg_rl = nc.dram_tensor("dbg_rl", [128, 512], F32, kind="ExternalOutput")
        dbg_m1 = nc.dram_tensor("dbg_m1", [128, 512], F32, kind="ExternalOutput")

    g_own = [nc.dram_tensor(f"g_own{l}", [npc, 64], F32) for l in range(L - 1)]
    g2 = [nc.dram_tensor(f"g2_{l}", [n_nodes_total, 64], F32,
                         addr_space="Shared") for l in range(L - 1)]
    rg = [list(range(ncores))]

    ntp = (HALFN + 127) // 128
    nfull = HALFN // 128
    rem = HALFN - nfull * 128

    with TileContext(nc) as tc:
        with (tc.tile_pool(name="const", bufs=1) as cpool,
              tc.tile_pool(name="big", bufs=1) as bigp,
              tc.tile_pool(name="asp", bufs=1) as asp,
              tc.tile_pool(name="gather", bufs=2) as gpool,
              tc.tile_pool(name="rank", bufs=2) as rkp,
              tc.tile_pool(name="work", bufs=3) as wkp,
              tc.tile_pool(name="psum", bufs=2, space="PSUM") as pp,
              tc.tile_pool(name="psum2", bufs=2, space="PSUM") as pp2,
              tc.tile_pool(name="psmall", bufs=4, space="PSUM") as pps):

            idx_sb = bigp.tile([128, S // 128], I32)
            nc.sync.dma_start(out=idx_sb[:], in_=idx_d[:])
            ident = cpool.tile([128, 128], F32)
            nc.sync.dma_start(out=ident[:], in_=ident_d[:])
            ident2 = cpool.tile([128, 128], F32R)
            nc.sync.dma_start(out=ident2[:], in_=ident2_d[:])
            l0w = cpool.tile([4, 128], F32R)
            nc.sync.dma_start(out=l0w[:], in_=l0w_d[:])
            eaw = cpool.tile([2, (L - 1) * 128], F32R)
            nc.sync.dma_start(out=eaw[:], in_=eaw_d[:])
            b1r = cpool.tile([128, L], F32)
            nc.sync.dma_start(out=b1r[:], in_=b1r_d[:])
            w2s = cpool.tile([128, L * 128], F32R)
            nc.sync.dma_start(out=w2s[:], in_=w2s_d[:])
            u1a = cpool.tile([128, L * 64], F32R)
            nc.sync.dma_start(out=u1a[:], in_=u1a_d[:])
            u1b = cpool.tile([128, L * 64], F32R)
            nc.sync.dma_start(out=u1b[:], in_=u1b_d[:])
            c1r = cpool.tile([128, L], F32)
            nc.sync.dma_start(out=c1r[:], in_=c1r_d[:])
            c2r = cpool.tile([128, L], F32)
            nc.sync.dma_start(out=c2r[:], in_=c2r_d[:])
            u2 = cpool.tile([64, L * 256], F32R)
            nc.sync.dma_start(out=u2[:], in_=u2_d[:])
            w1n = cpool.tile([128, (L - 1) * 128], F32R)
            nc.sync.dma_start(out=w1n[:], in_=w1n_d[:])
            fcw = cpool.tile([128, 2], F32R)
            nc.sync.dma_start(out=fcw[:], in_=fcw_d[:])
            fcb = cpool.tile([1, 1], F32)
            nc.sync.dma_start(out=fcb[:], in_=fcb_d[:])
            emb2 = cpool.tile([2, 256], F32R)
            nc.sync.dma_start(out=emb2[:], in_=emb2_d[:])

            hts = [bigp.tile([128, HALFN], F32R, tag=f"hts{i}", name=f"hts{i}")
                   for i in range(2)]

            # ---- embed h1 into hts[0] ----
            for c0 in range(0, HALFN, 512):
                cw = min(512, HALFN - c0)
                for half in range(2):
                    xt = rkp.tile([2, 512], F32R, tag="xt", name="xt")
                    nc.sync.dma_start(
                        out=xt[:, :cw],
                        in_=xo2_d[:, half * HALFN + c0:half * HALFN + c0 + cw])
                    ps = pps.tile([128, 512], F32, tag="sp", name="ps")
                    po = 64 if half == 0 else 0
                    nc.tensor.matmul(
                        out=ps[:, :cw],
                        lhsT=emb2[:, half * 128:(half + 1) * 128],
                        rhs=xt[:, :cw],
                        start=True, stop=True)
                    nc.scalar.activation(
                        out=hts[0][po:po + 64, c0:c0 + cw],
                        in_=ps[po:po + 64, :cw], func=AF.Identity)

            if debug:
                nc.sync.dma_start(out=dbg_h1[:], in_=hts[0][:].bitcast(F32))
            cur = 0
            for l in range(L):
                htile = hts[cur]
                aggr = asp.tile([128, max(n_aggr, ntp * 2 * 64)], F32R,
                                tag="as", name="aggr")
                # ---------- edge phase ----------
                nodecol = 0
                colhalf = 0
                for (u0, nu, tlist) in groups:
                    if l > 0:
                        gt = gpool.tile([128, GT * 64], F32, tag="g", name="gt")
                        for uu in range(nu):
                            nc.gpsimd.indirect_dma_start(
                                out=gt[:, uu * 64:(uu + 1) * 64],
                                out_offset=None,
                                in_=g2[l - 1][:],
                                in_offset=bass.IndirectOffsetOnAxis(
                                    ap=idx_sb[:, u0 + uu:u0 + uu + 1], axis=0))
                        if debug and l == 1 and u0 == 0:
                            nc.sync.dma_start(out=dbg_gt[:, :nu * 64],
                                              in_=gt[:, :nu * 64])
                    ccount = nu * 64
                    if l == 0:
                        rkt = rkp.tile([4, GT * 64], F32R, tag="rk", name="rkt")
                        nc.sync.dma_start(out=rkt[:, :ccount],
                                          in_=xe4_d[:, colhalf:colhalf + ccount])
                    else:
                        rkt = rkp.tile([2, GT * 64], F32R, tag="rk", name="rkt")
                        nc.sync.dma_start(out=rkt[:, :ccount],
                                          in_=ea2_d[:, colhalf:colhalf + ccount])
                    goff = 0
                    for ti in tlist:
                        K, w = tiles[ti]
                        c = w // 128
                        m1 = pp.tile([128, 512], F32, tag="m1", name="m1")
                        if l > 0:
                            nc.tensor.matmul(
                                out=m1[:, :w],
                                lhsT=eaw[:, (l - 1) * 128:l * 128],
                                rhs=rkt[0:2, goff * 64:goff * 64 + w],
                                start=True, stop=False, skip_group_check=True)
                            for ch in range(c):
                                nc.tensor.matmul(
                                    out=m1[:, ch * 128:(ch + 1) * 128],
                                    lhsT=gt[:, (goff + 2 * ch) * 64:
                                            (goff + 2 * ch + 2) * 64],
                                    rhs=ident[:],
                                    is_transpose=True, start=False,
                                    stop=(ch == c - 1),
                                    skip_group_check=True)
                        else:
                            nc.tensor.matmul(
                                out=m1[:, :w],
                                lhsT=l0w[:],
                                rhs=rkt[0:4, goff * 64:goff * 64 + w],
                                start=True, stop=True)
                        relu = wkp.tile([128, 512], F32R, tag="relu", name="relu")
                        nc.scalar.activation(out=relu[:, :w], in_=m1[:, :w],
                                             func=AF.Relu, bias=b1r[:, l:l + 1])
                        if debug and l == 1 and ti == tlist[0] and u0 == 0:
                            nc.sync.dma_start(out=dbg_rl[:, :w],
                                              in_=relu[:, :w].bitcast(F32))
                            m1c_ = wkp.tile([128, 512], F32, tag="relu", name="m1c_")
                            nc.scalar.activation(out=m1c_[:, :w], in_=m1[:, :w],
                                                 func=AF.Identity)
                            nc.sync.dma_start(out=dbg_m1[:, :w], in_=m1c_[:, :w])
                        msg = pp2.tile([128, 512], F32, tag="msg", name="msg")
                        nc.tensor.matmul(
                            out=msg[:, :w],
                            lhsT=w2s[:, l * 128:(l + 1) * 128],
                            rhs=relu[:, :w],
                            start=True, stop=True)
                        nn_ = w // K
                        nc.vector.tensor_reduce(
                            out=aggr[:, nodecol:nodecol + nn_],
                            in_=msg[:, :w].rearrange("p (n k) -> p n k", k=K),
                            axis=mybir.AxisListType.X, op=ALU.min)
                        nodecol += nn_
                        goff += 2 * c
                    colhalf += ccount
                assert nodecol == n_aggr
                if debug and l == 0:
                    nc.sync.dma_start(out=dbg_ag[:], in_=aggr[:, :n_aggr].bitcast(F32))
                if debug and l == 1:
                    nc.sync.dma_start(out=dbg_ag1[:], in_=aggr[:, :n_aggr].bitcast(F32))
                # ---------- update phase ----------
                nxt = 1 - cur
                hnew = hts[nxt]
                for c0 in range(0, HALFN, 512):
                    cw = min(512, HALFN - c0)
                    for half in range(2):
                        u = wkp.tile([128, 512], F32R, tag="u", name="u")
                        if half == 0:
                            nc.sync.dma_start(out=u[0:64, :cw],
                                              in_=aggr[0:64, c0:c0 + cw])
                            nc.sync.dma_start(out=u[64:128, :cw],
                                              in_=htile[64:128, c0:c0 + cw])
                            lhs1 = u1a
                        else:
                            nc.sync.dma_start(out=u[0:64, :cw],
                                              in_=htile[0:64, c0:c0 + cw])
                            nc.sync.dma_start(out=u[64:128, :cw],
                                              in_=aggr[64:128, c0:c0 + cw])
                            lhs1 = u1b
                        r1p = pps.tile([64, 512], F32, tag="sp", name="r1p")
                        nc.tensor.matmul(
                            out=r1p[:, :cw],
                            lhsT=lhs1[:, l * 64:(l + 1) * 64],
                            rhs=u[:, :cw],
                            start=True, stop=True)
                        r1 = wkp.tile([64, 512], F32R, tag="r1", name="r1")
                        nc.scalar.activation(out=r1[:, :cw], in_=r1p[:, :cw],
                                             func=AF.Relu, bias=c1r[0:64, l:l + 1])
                        hp = pps.tile([128, 512], F32, tag="sp", name="hp")
                        po = 64 if half == 0 else 0
                        nc.tensor.matmul(out=hp[:, :cw],
                                         lhsT=u2[:, l * 256 + half * 128:
                                                 l * 256 + (half + 1) * 128],
                                         rhs=r1[:, :cw],
                                         start=True, stop=True)
                        nc.scalar.activation(out=hnew[po:po + 64, c0:c0 + cw],
                                             in_=hp[po:po + 64, :cw],
                                             func=AF.Identity,
                                             bias=c2r[po:po + 64, l:l + 1])
                cur = nxt
                if debug and l == 0:
                    nc.sync.dma_start(out=dbg_h2[:], in_=hnew[:].bitcast(F32))
                # ---------- next-layer gather source / final fc ----------
                if l < L - 1:
                    stage = asp.tile([128, max(n_aggr, ntp * 2 * 64)], F32,
                                     tag="as", name="stage")
                    for c0 in range(0, HALFN, 512):
                        cw = min(512, HALFN - c0)
                        gs = wkp.tile([128, 512], F32R, tag="gs", name="gs")
                        gp_ = pps.tile([128, 512], F32, tag="sp", name="gp_")
                        nc.tensor.matmul(
                            out=gp_[:, :cw],
                            lhsT=w1n[:, l * 128:(l + 1) * 128],
                            rhs=hnew[:, c0:c0 + cw],
                            start=True, stop=True)
                        nc.scalar.activation(out=gs[:, :cw], in_=gp_[:, :cw],
                                             func=AF.Identity)
                        for sub in range(0, cw, 128):
                            sw = min(128, cw - sub)
                            t = (c0 + sub) // 128
                            for half in range(2):
                                tp = pps.tile([128, 64], F32, tag="sp", name="tp")
                                nc.tensor.matmul(
                                    out=tp[:sw, :],
                                    lhsT=gs[:, sub:sub + sw],
                                    rhs=ident2[:, half * 64:(half + 1) * 64],
                                    start=True, stop=True)
                                st_col = (half * ntp + t) * 64
                                nc.vector.tensor_copy(
                                    out=stage[:sw, st_col:st_col + 64],
                                    in_=tp[:sw, :])
                    for half in range(2):
                        off = half * ntp * 64
                        nc.sync.dma_start(
                            out=g_own[l][half * HALFN:
                                         half * HALFN + nfull * 128, :]
                                .rearrange("(t p) f -> p t f", p=128),
                            in_=stage[:, off:off + nfull * 64]
                                .rearrange("p (t f) -> p t f", f=64))
                        if rem:
                            nc.sync.dma_start(
                                out=g_own[l][half * HALFN + nfull * 128:
                                             half * HALFN + HALFN, :],
                                in_=stage[0:rem,
                                          off + nfull * 64:off + ntp * 64])
                    if debug and l == 0:
                        nc.sync.dma_start(out=dbg_go[:], in_=g_own[l][:])
                    nc.gpsimd.collective_compute(
                        "AllGather", ALU.bypass,
                        ins=[g_own[l][:]], outs=[g2[l][:]],
                        replica_groups=rg)
                    if debug and l == 0:
                        nc.sync.dma_start(out=dbg_g2[:], in_=g2[l][:])
                else:
                    hfin = hts[cur]
                    for c0 in range(0, HALFN, 512):
                        cw = min(512, HALFN - c0)
                        for half in range(2):
                            po = 64 if half == 0 else 0
                            fp_ = pps.tile([1, 512], F32, tag="sp", name="fp_")
                            nc.tensor.matmul(
                                out=fp_[:, :cw],
                                lhsT=fcw[:, half:half + 1],
                                rhs=hfin[:, c0:c0 + cw],
                                start=True, stop=True)
                            ob = wkp.tile([1, 512], F32, tag="ob", name="ob")
                            nc.scalar.activation(out=ob[:, :cw], in_=fp_[:, :cw],
                                                 func=AF.Identity, bias=fcb[:])
                            nc.sync.dma_start(
                                out=out_d[:, half * HALFN + c0:
                                          half * HALFN + c0 + cw],
                                in_=ob[:, :cw])
    return nc


# ===================== walrus wait-cap workaround =====================
"""Workarounds for this image's walrus build, which caps sync waits at 1 per
instruction (2 for EventSemaphore). Tile emits instructions with more. After
the TileContext exits, split excess waits onto same-engine nop instructions
inserted immediately before the over-subscribed instruction."""

_ctr = [0]


def legalize_waits(nc):
    m = nc.m
    for f in m.functions:
        for b in f.blocks:
            newinsts = []
            for inst in b.instructions:
                si = inst.sync_info
                cap = 2 if isinstance(inst, mybir.InstEventSemaphore) else 1
                if si is not None and si.on_wait and len(si.on_wait) > cap:
                    waits = list(si.on_wait)
                    si.on_wait = waits[:cap]
                    extra = waits[cap:]
                    for i in range(0, len(extra)):
                        _ctr[0] += 1
                        nop = mybir.InstNoOp(
                            name=f"waitsplit-{_ctr[0]}",
                            engine=inst.engine,
                            ins=[], outs=[],
                            sync_info=mybir.SyncInfo(
                                on_update=[], on_wait=[extra[i]]),
                        )
                        newinsts.append(nop)
                newinsts.append(inst)
            b.instructions = newinsts


def apply_patch():
    # kept for backwards compat; legalize_waits covers the drain case too.
    pass


# ===================== entry point =====================
def kernel(**inputs):
    inputs = {k: np.asarray(v) for k, v in inputs.items()}
    n = inputs["x"].shape[0]
    ncores = 8
    npc = n // ncores
    pp = prep(inputs["edge_index"], inputs["edge_attr"], inputs["x"],
              n=n, ncores=ncores)
    geom = {"S": pp["S"], "S_half_cols": pp["S_half_cols"],
            "tiles": pp["tiles"], "n_aggr_cols": pp["n_aggr_cols"]}
    L = inputs["msg_w1"].shape[0]
    nc = build_nc(geom, L=L, n_nodes_total=n, npc=npc, ncores=ncores)
    legalize_waits(nc)
    in_maps = make_inmaps(inputs, pp, npc)
    from concourse.bass_utils import run_bass_kernel_spmd
    res = run_bass_kernel_spmd(nc, in_maps, core_ids=list(range(ncores)))
    out = assemble_output(res.results, pp, n, npc)
    return out.astype(np.float32)



# revision 4
# speedup vs baseline: 1.0523x; 1.0523x over previous
"""Trainium2 Bass kernel for nn_DijkstraGNN (5-layer scatter-min GNN).

Self-contained: host-side graph preprocessing (dst-sharded, degree-bucketed
duplicate-padding), an SPMD Bass/Tile program for 8 NeuronCores, and output
assembly. Called as kernel(**inputs) with the full unsharded inputs.
"""
import sys
sys.path.insert(0, "/opt/trn_rl_repo")
import numpy as np
import concourse.bass as bass
import concourse.mybir as mybir
from concourse.tile import TileContext

# ===================== host-side preprocessing =====================
"""Host-side graph preprocessing for the dst-sharded, bucket-padded GNN kernel.

Layout (per core, 12500 owned nodes, halves A/B of 6250):
- Edge "slots" live at positions (p=partition, u=column) of the gather tile.
  Transpose chunk a covers u in {2a, 2a+1}; u even -> half A (psum rows 0-63),
  u odd -> half B (rows 64-127). PSUM column (within half) = 128*(u//2) + p... i.e.
  col c <-> (p = c % 128, u = 2*(c//128) + half).
- A node's K slots occupy K consecutive psum columns; runs never straddle a
  psum tile. Tiles are uniform-K, width = 384 cols (K % 3 == 0) else 512.
- K in {4,8,12,16,24,32,48,64,96,128}. Buckets laid out K-DESCENDING; the final
  (K=4) bucket is ghost-padded at the end so per-half chunk counts are integral.
- aggr col order defines the node permutation: half A cols = local rows 0..6249,
  half B = 6250..12499. Geometry (tiles list) is identical across cores.
"""

N = 100000
NCORES = 8
NODES_PER_CORE = N // NCORES      # 12500
HALF = NODES_PER_CORE // 2        # 6250
H = 64
CHUNK = 128

KS_DESC = [128, 96, 64, 48, 32, 24, 16, 12, 8]
K_MIN = KS_DESC[-1]


def width_of(K):
    return 384 if K % 3 == 0 else 512


def k_of_degree(d):
    for K in reversed(KS_DESC):           # ascending
        if d <= K:
            return K
    raise ValueError(f"degree {d} > 128 unsupported")


def prep(edge_index, edge_attr, x, n=None, ncores=None):
    global N, NCORES, NODES_PER_CORE, HALF
    if n is not None:
        N = n
    if ncores is not None:
        NCORES = ncores
    NODES_PER_CORE = N // NCORES
    HALF = NODES_PER_CORE // 2
    src = np.asarray(edge_index[0]).astype(np.int64).reshape(-1)
    dst = np.asarray(edge_index[1]).astype(np.int64).reshape(-1)
    ea = np.asarray(edge_attr).reshape(-1).astype(np.float32)
    xv = np.asarray(x).reshape(-1).astype(np.float32)

    deg = np.bincount(dst, minlength=N)
    assert deg.min() >= 1, "degree-0 nodes unsupported (none in this dataset)"

    order = np.argsort(dst, kind="stable")
    ptr = np.zeros(N + 1, dtype=np.int64)
    np.cumsum(deg, out=ptr[1:])

    K_node = np.array([k_of_degree(d) for d in deg], dtype=np.int64)

    # --- equalize bucket node counts across cores (K descending) ---
    # C[r][i] = nodes of core r with K_node >= KS_DESC[i]  (cumulative)
    Cs = np.zeros((NCORES, len(KS_DESC)), dtype=np.int64)
    for r in range(NCORES):
        kn = K_node[r * NODES_PER_CORE:(r + 1) * NODES_PER_CORE]
        for i, K in enumerate(KS_DESC):
            Cs[r, i] = (kn >= K).sum()
    maxreq = Cs.max(axis=0)

    target = {}
    cum = 0
    for i, K in enumerate(KS_DESC[:-1]):
        per_half_nodes_unit = 1
        # per-half col count must be a multiple of CHUNK => count*K/2 % 128 == 0
        unit = 2 * CHUNK // np.gcd(K, 2 * CHUNK)          # count unit
        unit = max(unit, 2)                               # even halves
        # also ensure per-half count tiles into uniform-K tiles exactly:
        w = width_of(K)
        unit2 = 2 * (w // K) // np.gcd(w // K, 1)
        # count/2 % (w/K) == 0  => count % (2*w/K) == 0 ; and chunk alignment
        # (count/2)*K % 128 == 0 is implied when w % 128 == 0 and count/2 % (w/K)==0
        unit = int(np.lcm(unit, 2 * (w // K)))
        raw = max(0, int(maxreq[i]) - cum)
        t = ((raw + unit - 1) // unit) * unit
        target[K] = t
        cum += t
    target[K_MIN] = NODES_PER_CORE - cum                  # ghost-padded below

    def feasible():
        # cumulative-from-top coverage must meet maxreq at every level
        c = 0
        for i, K in enumerate(KS_DESC):
            c += target.get(K, 0)
            if c < maxreq[i]:
                return False
        return all(v >= 0 for v in target.values())

    # rounding may have over-allocated; shrink upper buckets (by their units)
    # while keeping coverage, until the last bucket is non-negative & feasible
    units = {}
    for K in KS_DESC[:-1]:
        w = width_of(K)
        units[K] = int(np.lcm(2 * CHUNK // np.gcd(K, 2 * CHUNK), 2 * (w // K)))
    guard = 0
    while not feasible():
        guard += 1
        assert guard < 10000, "bucket balancing failed"
        fixed = False
        for K in KS_DESC[-2::-1]:                 # ascending, skip K_MIN
            if target.get(K, 0) >= units[K]:
                target[K] -= units[K]
                target[K_MIN] += units[K]
                if feasible():
                    fixed = True
                    break
                # revert if this broke coverage at upper levels
                target[K] += units[K]
                target[K_MIN] -= units[K]
        if not fixed:
            raise RuntimeError("could not balance bucket targets")
    assert sum(target.values()) == NODES_PER_CORE

    # ghost pad for the last bucket: per-half COLUMN count must be a multiple
    # of CHUNK=128 (transpose granularity).
    c4 = target[K_MIN]
    assert c4 % 2 == 0, "last bucket count must be even"
    ghosts_half = ((-(c4 // 2) * K_MIN) % CHUNK) // K_MIN  # ghost nodes per half
    assert ((c4 // 2) + ghosts_half) * K_MIN % CHUNK == 0
    # --- tiles (uniform geometry across cores) ---
    tiles = []            # (K, ncols) per psum tile, ncols = per-half cols
    chunk_plan = []       # (K, per-half cols) per bucket, K descending
    for K in KS_DESC:
        t = target.get(K, 0)
        if K == K_MIN:
            cols = (t // 2 + ghosts_half) * K
        else:
            cols = t // 2 * K
        if cols == 0:
            continue
        chunk_plan.append((K, cols))
        w = width_of(K)
        nfull, rem = divmod(cols, w)
        tiles.extend([(K, w)] * nfull)
        if rem:
            assert rem % CHUNK == 0
            tiles.append((K, rem))
    S_half_cols = sum(c for _, c in chunk_plan)
    assert S_half_cols % CHUNK == 0
    S = 2 * S_half_cols

    perm = np.empty(N, dtype=np.int64)
    cores = []
    rng = np.random.default_rng(0)
    for r in range(NCORES):
        nodes = np.arange(r * NODES_PER_CORE, (r + 1) * NODES_PER_CORE)
        ksort = np.argsort(-K_node[nodes], kind="stable")  # descending K
        nodes = nodes[ksort]
        pos = 0
        slot_src = np.zeros((128, S // 128), dtype=np.int32)
        ea_rank = np.zeros((2, S_half_cols), dtype=np.float32)
        x_rank = np.zeros((2, S_half_cols), dtype=np.float32)
        permA, permB = [], []
        colbase = 0
        for K, cols in chunk_plan:
            nreal = target[K] // 2                 # real nodes per half
            for h_i in range(2):
                sel = nodes[pos + h_i * nreal: pos + (h_i + 1) * nreal]
                plist = permA if h_i == 0 else permB
                plist.extend(sel.tolist())
                # vectorized slot fill for this (bucket, half)
                if len(sel) == 0 and K != 4:
                    continue
                # edge ids per node, duplicate-padded to K
                eidx = np.zeros((max(len(sel), 1), K), dtype=np.int64)
                for j, node in enumerate(sel):
                    es = order[ptr[node]:ptr[node + 1]]
                    eidx[j] = np.resize(es, K)
                if len(sel) == 0:
                    eidx = eidx[:0]
                nghost = (cols - nreal * K) // K if K == K_MIN else 0
                if nghost:
                    ghost = np.zeros((nghost, K), dtype=np.int64)
                    ghost[:] = order[0]            # arbitrary real edge
                    eidx = np.concatenate([eidx, ghost], axis=0) if len(eidx) \
                        else ghost
                flat = eidx.reshape(-1)            # per-half col order
                cvec = colbase + np.arange(len(flat))
                pvec = cvec % CHUNK
                uvec = 2 * (cvec // CHUNK) + h_i
                slot_src[pvec, uvec] = src[flat]   # orig node id; remapped below
                ea_rank[h_i, colbase:colbase + len(flat)] = ea[flat]
                x_rank[h_i, colbase:colbase + len(flat)] = xv[src[flat]]
            pos += target[K]
            colbase += cols
        assert pos == NODES_PER_CORE and colbase == S_half_cols
        assert len(permA) == len(permB) == HALF
        perm_local = np.array(permA + permB, dtype=np.int64)
        perm[r * NODES_PER_CORE:(r + 1) * NODES_PER_CORE] = perm_local
        cores.append({"perm_local": perm_local, "slot_src": slot_src,
                      "ea_rank": ea_rank, "x_rank": x_rank})

    inv_perm = np.empty(N, dtype=np.int64)
    inv_perm[perm] = np.arange(N)
    # remap slot_src from orig node ids to permuted row ids
    for c in cores:
        c["slot_src"] = inv_perm[c["slot_src"]].astype(np.int32)

    return {"tiles": tiles, "S": S, "chunk_plan": chunk_plan, "cores": cores,
            "perm": perm, "inv_perm": inv_perm, "deg": deg,
            "S_half_cols": S_half_cols,
            "n_aggr_cols": sum(c // K for K, c in chunk_plan)}


# ===================== input packing =====================
"""Host-side input packing shared by test and kernel.py."""

H = 64


def make_inmaps(inputs, pp, npc):
    """Build per-core in_maps for the bass kernel."""
    L = inputs["msg_w1"].shape[0]
    W1 = np.asarray(inputs["msg_w1"], np.float32)       # [L, 65, 64]
    b1 = np.asarray(inputs["msg_b1"], np.float32)
    W2 = np.asarray(inputs["msg_w2"], np.float32)
    b2 = np.asarray(inputs["msg_b2"], np.float32)
    U1 = np.asarray(inputs["upd_w1"], np.float32)       # [L, 128, 64]
    c1 = np.asarray(inputs["upd_b1"], np.float32)
    U2 = np.asarray(inputs["upd_w2"], np.float32)
    c2 = np.asarray(inputs["upd_b2"], np.float32)
    emb_w = np.asarray(inputs["emb_w"], np.float32)     # [1, 64]
    emb_b = np.asarray(inputs["emb_b"], np.float32)     # [64]
    fc_w = np.asarray(inputs["fc_w"], np.float32)       # [64, 1]
    fc_b = np.asarray(inputs["fc_b"], np.float32)       # [1]
    xv = np.asarray(inputs["x"], np.float32).reshape(-1)

    W1a = W1[:, :H]                                     # [L, 64, 64]
    w1b = W1[:, H]                                      # [L, 64]
    alpha = emb_w[0] @ W1a[0]
    beta = emb_b @ W1a[0] + b1[0]

    l0w = np.zeros((4, 128), np.float32)
    l0w[0, :64] = alpha; l0w[1, :64] = w1b[0]
    l0w[2, 64:] = alpha; l0w[3, 64:] = w1b[0]
    eaw = np.zeros((L - 1, 2, 128), np.float32)
    for l in range(1, L):
        eaw[l - 1, 0, :64] = w1b[l]
        eaw[l - 1, 1, 64:] = w1b[l]
    b1r = np.zeros((L, 128, 1), np.float32)
    b1r[0, :, 0] = np.tile(beta, 2)
    for l in range(1, L):
        b1r[l, :, 0] = np.tile(b1[l], 2)
    w2s = np.zeros((L, 128, 128), np.float32)           # block-diag W2
    w2s[:, :64, :64] = W2
    w2s[:, 64:, 64:] = W2
    u1a = np.concatenate([U1[:, H:], U1[:, :H]], axis=1)
    u1b = U1
    c1p = c1 + np.einsum("lk,lko->lo", b2, U1[:, H:])
    c1r = np.tile(c1p[:, :, None], (1, 2, 1)).reshape(L, 128, 1)
    c2r = np.tile(c2[:, :, None], (1, 2, 1)).reshape(L, 128, 1)
    w1n = np.zeros((L - 1, 128, 128), np.float32)       # anti-block W1a_{l+1}
    w1n[:, 64:, :64] = W1a[1:]                          # A: h@64-127 -> gs@0-63
    w1n[:, :64, 64:] = W1a[1:]                          # B: h@0-63 -> gs@64-127
    fcw = np.zeros((128, 2), np.float32)
    fcw[64:, 0] = fc_w[:, 0]                            # A variant
    fcw[:64, 1] = fc_w[:, 0]                            # B variant
    fcb = fc_b.reshape(1, 1)
    emb2 = np.zeros((2, 256), np.float32)               # [2, 2*128] variants
    emb2[0, 64:128] = emb_w[0]; emb2[1, 64:128] = emb_b     # A -> rows 64-127
    emb2[0, 128:192] = emb_w[0]; emb2[1, 128:192] = emb_b   # B -> rows 0-63
    ident = np.eye(128, dtype=np.float32)
    u2v = np.zeros((L, 64, 256), np.float32)
    u2v[:, :, 64:128] = U2                              # A -> out rows 64-127
    u2v[:, :, 128:192] = U2                             # B -> out rows 0-63

    def pack(a):      # [L, P, C] -> [P, L*C]
        return np.ascontiguousarray(a.transpose(1, 0, 2).reshape(a.shape[1], -1))

    shared = dict(l0w=l0w, eaw=pack(eaw), b1r=pack(b1r), w2s=pack(w2s),
                  u1a=pack(u1a), u1b=pack(u1b), c1r=pack(c1r), c2r=pack(c2r),
                  u2=pack(u2v), w1n=pack(w1n), fcw=fcw, fcb=fcb,
                  emb2=emb2, ident=ident,
                  ident2=np.concatenate(
                      [np.concatenate([np.eye(64), np.zeros((64, 64))], 0),
                       np.concatenate([np.zeros((64, 64)), np.eye(64)], 0)],
                      axis=1).astype(np.float32))

    in_maps = []
    for r, core in enumerate(pp["cores"]):
        m = dict(shared)
        m["idx"] = core["slot_src"]
        m["ea2"] = core["ea_rank"]
        m["xe4"] = np.concatenate([core["x_rank"][0:1], core["ea_rank"][0:1],
                                   core["x_rank"][1:2], core["ea_rank"][1:2]],
                                  axis=0)
        xo = xv[core["perm_local"]]
        m["xo2"] = np.stack([xo, np.ones(npc, np.float32)], axis=0)
        in_maps.append(m)
    return in_maps


def assemble_output(results, pp, n, npc):
    out = np.zeros(n, dtype=np.float32)
    for r in range(len(results)):
        loc = results[r]["out_y"].reshape(-1)
        out[pp["perm"][r * npc:(r + 1) * npc]] = loc
    return out


# ===================== device program =====================
"""Bass device-program builder for the DijkstraGNN kernel."""

F32 = mybir.dt.float32
F32R = mybir.dt.float32r
I32 = mybir.dt.int32
AF = mybir.ActivationFunctionType
ALU = mybir.AluOpType


def build_nc(geom, L=5, n_nodes_total=100000, npc=12500, ncores=8, debug=False):
    S = geom["S"]
    S_half = geom["S_half_cols"]
    tiles = geom["tiles"]          # list of (K, w) psum tiles, uniform-K
    HALFN = npc // 2
    n_aggr = geom["n_aggr_cols"]

    GT = 48                        # idx-cols per gather group
    groups = []                    # [ucol0, nucols, [tile indices]]
    cur = None
    ucum = 0
    for ti, (K, w) in enumerate(tiles):
        uc = w // 64
        if cur is None or cur[1] + uc > GT:
            cur = [ucum, 0, []]
            groups.append(cur)
        cur[1] += uc
        cur[2].append(ti)
        ucum += uc
    assert ucum == S // 128

    nc = bass.Bass(num_swdge_queues=4)
    idx_d = nc.dram_tensor("idx", [128, S // 128], I32, kind="ExternalInput")
    ea2_d = nc.dram_tensor("ea2", [2, S_half], F32R, kind="ExternalInput")
    xe4_d = nc.dram_tensor("xe4", [4, S_half], F32R, kind="ExternalInput")
    xo2_d = nc.dram_tensor("xo2", [2, npc], F32R, kind="ExternalInput")
    emb2_d = nc.dram_tensor("emb2", [2, 256], F32R, kind="ExternalInput")
    l0w_d = nc.dram_tensor("l0w", [4, 128], F32R, kind="ExternalInput")
    eaw_d = nc.dram_tensor("eaw", [2, (L - 1) * 128], F32R, kind="ExternalInput")
    b1r_d = nc.dram_tensor("b1r", [128, L], F32, kind="ExternalInput")
    w2s_d = nc.dram_tensor("w2s", [128, L * 128], F32R, kind="ExternalInput")
    u1a_d = nc.dram_tensor("u1a", [128, L * 64], F32R, kind="ExternalInput")
    u1b_d = nc.dram_tensor("u1b", [128, L * 64], F32R, kind="ExternalInput")
    c1r_d = nc.dram_tensor("c1r", [128, L], F32, kind="ExternalInput")
    c2r_d = nc.dram_tensor("c2r", [128, L], F32, kind="ExternalInput")
    u2_d = nc.dram_tensor("u2", [64, L * 256], F32R, kind="ExternalInput")
    w1n_d = nc.dram_tensor("w1n", [128, (L - 1) * 128], F32R, kind="ExternalInput")
    fcw_d = nc.dram_tensor("fcw", [128, 2], F32R, kind="ExternalInput")
    fcb_d = nc.dram_tensor("fcb", [1, 1], F32, kind="ExternalInput")
    ident_d = nc.dram_tensor("ident", [128, 128], F32, kind="ExternalInput")
    ident2_d = nc.dram_tensor("ident2", [128, 128], F32R, kind="ExternalInput")
    out_d = nc.dram_tensor("out_y", [1, npc], F32, kind="ExternalOutput")
    if debug:
        dbg_h1 = nc.dram_tensor("dbg_h1", [128, npc // 2], F32, kind="ExternalOutput")
        dbg_ag = nc.dram_tensor("dbg_ag", [128, geom["n_aggr_cols"]], F32, kind="ExternalOutput")
        dbg_h2 = nc.dram_tensor("dbg_h2", [128, npc // 2], F32, kind="ExternalOutput")
        dbg_go = nc.dram_tensor("dbg_go", [npc, 64], F32, kind="ExternalOutput")
        dbg_g2 = nc.dram_tensor("dbg_g2", [n_nodes_total, 64], F32, kind="ExternalOutput")
        dbg_ag1 = nc.dram_tensor("dbg_ag1", [128, geom["n_aggr_cols"]], F32, kind="ExternalOutput")
        dbg_gt = nc.dram_tensor("dbg_gt", [128, GT * 64], F32, kind="ExternalOutput")
        dbg_rl = nc.dram_tensor("dbg_rl", [128, 512], F32, kind="ExternalOutput")
        dbg_m1 = nc.dram_tensor("dbg_m1", [128, 512], F32, kind="ExternalOutput")

    g_own = [nc.dram_tensor(f"g_own{l}", [npc, 64], F32) for l in range(L - 1)]
    g2 = [nc.dram_tensor(f"g2_{l}", [n_nodes_total, 64], F32,
                         addr_space="Shared") for l in range(L - 1)]
    rg = [list(range(ncores))]

    ntp = (HALFN + 127) // 128
    nfull = HALFN // 128
    rem = HALFN - nfull * 128

    with TileContext(nc) as tc:
        with (tc.tile_pool(name="const", bufs=1) as cpool,
              tc.tile_pool(name="big", bufs=1) as bigp,
              tc.tile_pool(name="asp", bufs=1) as asp,
              tc.tile_pool(name="gather", bufs=2) as gpool,
              tc.tile_pool(name="rank", bufs=2) as rkp,
              tc.tile_pool(name="work", bufs=3) as wkp,
              tc.tile_pool(name="psum", bufs=2, space="PSUM") as pp,
              tc.tile_pool(name="psum2", bufs=2, space="PSUM") as pp2,
              tc.tile_pool(name="psmall", bufs=4, space="PSUM") as pps):

            idx_sb = bigp.tile([128, S // 128], I32)
            nc.sync.dma_start(out=idx_sb[:], in_=idx_d[:])
            ident = cpool.tile([128, 128], F32)
            nc.sync.dma_start(out=ident[:], in_=ident_d[:])
            ident2 = cpool.tile([128, 128], F32R)
            nc.sync.dma_start(out=ident2[:], in_=ident2_d[:])
            l0w = cpool.tile([4, 128], F32R)
            nc.sync.dma_start(out=l0w[:], in_=l0w_d[:])
            eaw = cpool.tile([2, (L - 1) * 128], F32R)
            nc.sync.dma_start(out=eaw[:], in_=eaw_d[:])
            b1r = cpool.tile([128, L], F32)
            nc.sync.dma_start(out=b1r[:], in_=b1r_d[:])
            w2s = cpool.tile([128, L * 128], F32R)
            nc.sync.dma_start(out=w2s[:], in_=w2s_d[:])
            u1a = cpool.tile([128, L * 64], F32R)
            nc.sync.dma_start(out=u1a[:], in_=u1a_d[:])
            u1b = cpool.tile([128, L * 64], F32R)
            nc.sync.dma_start(out=u1b[:], in_=u1b_d[:])
            c1r = cpool.tile([128, L], F32)
            nc.sync.dma_start(out=c1r[:], in_=c1r_d[:])
            c2r = cpool.tile([128, L], F32)
            nc.sync.dma_start(out=c2r[:], in_=c2r_d[:])
            u2 = cpool.tile([64, L * 256], F32R)
            nc.sync.dma_start(out=u2[:], in_=u2_d[:])
            w1n = cpool.tile([128, (L - 1) * 128], F32R)
            nc.sync.dma_start(out=w1n[:], in_=w1n_d[:])
            fcw = cpool.tile([128, 2], F32R)
            nc.sync.dma_start(out=fcw[:], in_=fcw_d[:])
            fcb = cpool.tile([1, 1], F32)
            nc.sync.dma_start(out=fcb[:], in_=fcb_d[:])
            emb2 = cpool.tile([2, 256], F32R)
            nc.sync.dma_start(out=emb2[:], in_=emb2_d[:])

            hts = [bigp.tile([128, HALFN], F32R, tag=f"hts{i}", name=f"hts{i}")
                   for i in range(2)]

            # ---- embed h1 into hts[0] ----
            for c0 in range(0, HALFN, 512):
                cw = min(512, HALFN - c0)
                for half in range(2):
                    xt = rkp.tile([2, 512], F32R, tag="xt", name="xt")
                    nc.sync.dma_start(
                        out=xt[:, :cw],
                        in_=xo2_d[:, half * HALFN + c0:half * HALFN + c0 + cw])
                    ps = pps.tile([128, 512], F32, tag="sp", name="ps")
                    po = 64 if half == 0 else 0
                    nc.tensor.matmul(
                        out=ps[:, :cw],
                        lhsT=emb2[:, half * 128:(half + 1) * 128],
                        rhs=xt[:, :cw],
                        start=True, stop=True)
                    nc.scalar.activation(
                        out=hts[0][po:po + 64, c0:c0 + cw],
                        in_=ps[po:po + 64, :cw], func=AF.Identity)

            if debug:
                nc.sync.dma_start(out=dbg_h1[:], in_=hts[0][:].bitcast(F32))
            cur = 0
            for l in range(L):
                htile = hts[cur]
                aggr = asp.tile([128, max(n_aggr, ntp * 2 * 64)], F32R,
                                tag="as", name="aggr")
                # ---------- edge phase ----------
                nodecol = 0
                colhalf = 0
                for (u0, nu, tlist) in groups:
                    if l > 0:
                        gt = gpool.tile([128, GT * 64], F32, tag="g", name="gt")
                        for uu in range(nu):
                            nc.gpsimd.indirect_dma_start(
                                out=gt[:, uu * 64:(uu + 1) * 64],
                                out_offset=None,
                                in_=g2[l - 1][:],
                                in_offset=bass.IndirectOffsetOnAxis(
                                    ap=idx_sb[:, u0 + uu:u0 + uu + 1], axis=0))
                        if debug and l == 1 and u0 == 0:
                            nc.sync.dma_start(out=dbg_gt[:, :nu * 64],
                                              in_=gt[:, :nu * 64])
                    ccount = nu * 64
                    if l == 0:
                        rkt = rkp.tile([4, GT * 64], F32R, tag="rk", name="rkt")
                        nc.sync.dma_start(out=rkt[:, :ccount],
                                          in_=xe4_d[:, colhalf:colhalf + ccount])
                    else:
                        rkt = rkp.tile([2, GT * 64], F32R, tag="rk", name="rkt")
                        nc.sync.dma_start(out=rkt[:, :ccount],
                                          in_=ea2_d[:, colhalf:colhalf + ccount])
                    goff = 0
                    for ti in tlist:
                        K, w = tiles[ti]
                        c = w // 128
                        m1 = pp.tile([128, 512], F32, tag="m1", name="m1")
                        if l > 0:
                            nc.tensor.matmul(
                                out=m1[:, :w],
                                lhsT=eaw[:, (l - 1) * 128:l * 128],
                                rhs=rkt[0:2, goff * 64:goff * 64 + w],
                                start=True, stop=False, skip_group_check=True)
                            for ch in range(c):
                                nc.tensor.matmul(
                                    out=m1[:, ch * 128:(ch + 1) * 128],
                                    lhsT=gt[:, (goff + 2 * ch) * 64:
                                            (goff + 2 * ch + 2) * 64],
                                    rhs=ident[:],
                                    is_transpose=True, start=False,
                                    stop=(ch == c - 1),
                                    skip_group_check=True)
                        else:
                            nc.tensor.matmul(
                                out=m1[:, :w],
                                lhsT=l0w[:],
                                rhs=rkt[0:4, goff * 64:goff * 64 + w],
                                start=True, stop=True)
                        relu = wkp.tile([128, 512], F32R, tag="relu", name="relu")
                        nc.scalar.activation(out=relu[:, :w], in_=m1[:, :w],
                                             func=AF.Relu, bias=b1r[:, l:l + 1])
                        if debug and l == 1 and ti == tlist[0] and u0 == 0:
                            nc.sync.dma_start(out=dbg_rl[:, :w],
                                              in_=relu[:, :w].bitcast(F32))
                            m1c_ = wkp.tile([128, 512], F32, tag="relu", name="m1c_")
                            nc.scalar.activation(out=m1c_[:, :w], in_=m1[:, :w],
                                                 func=AF.Identity)
                            nc.sync.dma_start(out=dbg_m1[:, :w], in_=m1c_[:, :w])
                        msg = pp2.tile([128, 512], F32, tag="msg", name="msg")
                        nc.tensor.matmul(
                            out=msg[:, :w],
                            lhsT=w2s[:, l * 128:(l + 1) * 128],
                            rhs=relu[:, :w],
                            start=True, stop=True)
                        nn_ = w // K
                        nc.vector.tensor_reduce(
                            out=aggr[:, nodecol:nodecol + nn_],
                            in_=msg[:, :w].rearrange("p (n k) -> p n k", k=K),
                            axis=mybir.AxisListType.X, op=ALU.min)
                        nodecol += nn_
                        goff += 2 * c
                    colhalf += ccount
                assert nodecol == n_aggr
                if debug and l == 0:
                    nc.sync.dma_start(out=dbg_ag[:], in_=aggr[:, :n_aggr].bitcast(F32))
                if debug and l == 1:
                    nc.sync.dma_start(out=dbg_ag1[:], in_=aggr[:, :n_aggr].bitcast(F32))
                # ---------- update phase ----------
                nxt = 1 - cur
                hnew = hts[nxt]
                for c0 in range(0, HALFN, 512):
                    cw = min(512, HALFN - c0)
                    for half in range(2):
                        u = wkp.tile([128, 512], F32R, tag="u", name="u")
                        if half == 0:
                            nc.sync.dma_start(out=u[0:64, :cw],
                                              in_=aggr[0:64, c0:c0 + cw])
                            nc.sync.dma_start(out=u[64:128, :cw],
                                              in_=htile[64:128, c0:c0 + cw])
                            lhs1 = u1a
                        else:
                            nc.sync.dma_start(out=u[0:64, :cw],
                                              in_=htile[0:64, c0:c0 + cw])
                            nc.sync.dma_start(out=u[64:128, :cw],
                                              in_=aggr[64:128, c0:c0 + cw])
                            lhs1 = u1b
                        r1p = pps.tile([64, 512], F32, tag="sp", name="r1p")
                        nc.tensor.matmul(
                            out=r1p[:, :cw],
                            lhsT=lhs1[:, l * 64:(l + 1) * 64],
                            rhs=u[:, :cw],
                            start=True, stop=True)
                        r1 = wkp.tile([64, 512], F32R, tag="r1", name="r1")
                        nc.scalar.activation(out=r1[:, :cw], in_=r1p[:, :cw],
                                             func=AF.Relu, bias=c1r[0:64, l:l + 1])
                        hp = pps.tile([128, 512], F32, tag="sp", name="hp")
                        po = 64 if half == 0 else 0
                        nc.tensor.matmul(out=hp[:, :cw],
                                         lhsT=u2[:, l * 256 + half * 128:
                                                 l * 256 + (half + 1) * 128],
                                         rhs=r1[:, :cw],
                                         start=True, stop=True)
                        nc.scalar.activation(out=hnew[po:po + 64, c0:c0 + cw],
                                             in_=hp[po:po + 64, :cw],
                                             func=AF.Identity,
                                             bias=c2r[po:po + 64, l:l + 1])
                cur = nxt
                if debug and l == 0:
                    nc.sync.dma_start(out=dbg_h2[:], in_=hnew[:].bitcast(F32))
                # ---------- next-layer gather source / final fc ----------
                if l < L - 1:
                    stage = asp.tile([128, max(n_aggr, ntp * 2 * 64)], F32,
                                     tag="as", name="stage")
                    for c0 in range(0, HALFN, 512):
                        cw = min(512, HALFN - c0)
                        gs = wkp.tile([128, 512], F32R, tag="gs", name="gs")
                        gp_ = pps.tile([128, 512], F32, tag="sp", name="gp_")
                        nc.tensor.matmul(
                            out=gp_[:, :cw],
                            lhsT=w1n[:, l * 128:(l + 1) * 128],
                            rhs=hnew[:, c0:c0 + cw],
                            start=True, stop=True)
                        nc.scalar.activation(out=gs[:, :cw], in_=gp_[:, :cw],
                                             func=AF.Identity)
                        for sub in range(0, cw, 128):
                            sw = min(128, cw - sub)
                            t = (c0 + sub) // 128
                            for half in range(2):
                                tp = pps.tile([128, 64], F32, tag="sp", name="tp")
                                nc.tensor.matmul(
                                    out=tp[:sw, :],
                                    lhsT=gs[:, sub:sub + sw],
                                    rhs=ident2[:, half * 64:(half + 1) * 64],
                                    start=True, stop=True)
                                st_col = (half * ntp + t) * 64
                                nc.vector.tensor_copy(
                                    out=stage[:sw, st_col:st_col + 64],
                                    in_=tp[:sw, :])
                    for half in range(2):
                        off = half * ntp * 64
                        nc.sync.dma_start(
                            out=g_own[l][half * HALFN:
                                         half * HALFN + nfull * 128, :]
                                .rearrange("(t p) f -> p t f", p=128),
                            in_=stage[:, off:off + nfull * 64]
                                .rearrange("p (t f) -> p t f", f=64))
                        if rem:
                            nc.sync.dma_start(
                                out=g_own[l][half * HALFN + nfull * 128:
                                             half * HALFN + HALFN, :],
                                in_=stage[0:rem,
                                          off + nfull * 64:off + ntp * 64])
                    if debug and l == 0:
                        nc.sync.dma_start(out=dbg_go[:], in_=g_own[l][:])
                    nc.gpsimd.collective_compute(
                        "AllGather", ALU.bypass,
                        ins=[g_own[l][:]], outs=[g2[l][:]],
                        replica_groups=rg)
                    if debug and l == 0:
                        nc.sync.dma_start(out=dbg_g2[:], in_=g2[l][:])
                else:
                    hfin = hts[cur]
                    for c0 in range(0, HALFN, 512):
                        cw = min(512, HALFN - c0)
                        for half in range(2):
                            po = 64 if half == 0 else 0
                            fp_ = pps.tile([1, 512], F32, tag="sp", name="fp_")
                            nc.tensor.matmul(
                                out=fp_[:, :cw],
                                lhsT=fcw[:, half:half + 1],
                                rhs=hfin[:, c0:c0 + cw],
                                start=True, stop=True)
                            ob = wkp.tile([1, 512], F32, tag="ob", name="ob")
                            nc.scalar.activation(out=ob[:, :cw], in_=fp_[:, :cw],
                                                 func=AF.Identity, bias=fcb[:])
                            nc.sync.dma_start(
                                out=out_d[:, half * HALFN + c0:
                                          half * HALFN + c0 + cw],
                                in_=ob[:, :cw])
    return nc


# ===================== walrus wait-cap workaround =====================
"""Workarounds for this image's walrus build, which caps sync waits at 1 per
instruction (2 for EventSemaphore). Tile emits instructions with more. After
the TileContext exits, split excess waits onto same-engine nop instructions
inserted immediately before the over-subscribed instruction."""

_ctr = [0]


def legalize_waits(nc):
    m = nc.m
    for f in m.functions:
        for b in f.blocks:
            newinsts = []
            for inst in b.instructions:
                si = inst.sync_info
                cap = 2 if isinstance(inst, mybir.InstEventSemaphore) else 1
                if si is not None and si.on_wait and len(si.on_wait) > cap:
                    waits = list(si.on_wait)
                    si.on_wait = waits[:cap]
                    extra = waits[cap:]
                    for i in range(0, len(extra)):
                        _ctr[0] += 1
                        nop = mybir.InstNoOp(
                            name=f"waitsplit-{_ctr[0]}",
                            engine=inst.engine,
                            ins=[], outs=[],
                            sync_info=mybir.SyncInfo(
                                on_update=[], on_wait=[extra[i]]),
                        )
                        newinsts.append(nop)
                newinsts.append(inst)
            b.instructions = newinsts


def apply_patch():
    # kept for backwards compat; legalize_waits covers the drain case too.
    pass


# ===================== entry point =====================
def kernel(**inputs):
    inputs = {k: np.asarray(v) for k, v in inputs.items()}
    n = inputs["x"].shape[0]
    ncores = 8
    npc = n // ncores
    pp = prep(inputs["edge_index"], inputs["edge_attr"], inputs["x"],
              n=n, ncores=ncores)
    geom = {"S": pp["S"], "S_half_cols": pp["S_half_cols"],
            "tiles": pp["tiles"], "n_aggr_cols": pp["n_aggr_cols"]}
    L = inputs["msg_w1"].shape[0]
    nc = build_nc(geom, L=L, n_nodes_total=n, npc=npc, ncores=ncores)
    legalize_waits(nc)
    in_maps = make_inmaps(inputs, pp, npc)
    from concourse.bass_utils import run_bass_kernel_spmd
    res = run_bass_kernel_spmd(nc, in_maps, core_ids=list(range(ncores)))
    out = assemble_output(res.results, pp, n, npc)
    return out.astype(np.float32)

